# revision 28
# baseline (speedup 1.0000x reference)
"""Attention-pooling kernel for TRN2 (8 NeuronCores, SPMD).

Problem: enc [S=8192, B=32, H=256] f32, hid [1, B, H] f32.
  scores = einsum('sbh,bh->bs'); w = softmax(scores, axis=s)
  ctx    = einsum('sbh,bs->bh')

Two builders are kept:
  _build_nc  — the original ones-column layout (enc cols = 257 per b; l
               rides the ctx matmul).
  _build_nc2 — flat [128, 32*256] fp16 tiles, l via a per-tile PE matmul
               with a ones lhsT into PSUM partition 1 (partition-disjoint
               chain), single ACT drain copy of PSUM rows 0..96.
               Score engine split is parameterized (n_ttr on DVE fused
               affine_mul_reduce at ~313ns/col, n_act on DVE-bulk-mul@2x
               + ACT copy-accumulate at ~134+~650ns/col); measured-
               balanced split is 21/11. Microbenches (microbench.py) show
               DVE fp16 tensor_mul runs in 2x mode (0.51ns/elem) but
               tensor_reduce and custom-DVE ops are 1x, Pool serializes
               with DVE (shared SBUF port), and per-core DMA caps at
               ~337GB/s regardless of queue count — so the kernel floor
               is the DVE/ACT score balance (~8.1us/tile), slightly above
               the 6.3us/tile DMA floor.

Sharding: S split into 8 contiguous 1024-row slices (one per core); softmax
is decomposed as per-core partial sums with a *fixed* exponent shift C:
  w_c = exp(scores_c - C);  l_c = sum_s w_c;  ctx_c = sum_s w_c * enc
  ctx = sum_c ctx_c / sum_c l_c
The shift C=64 keeps exp in f32 range for this problem's score magnitudes
(max |score| ~ 76; exp(76-64)=e^12 ~ 1.6e5, far below f32 max) and cancels
exactly in the final division, so no cross-core max pass is needed.

The problem is HBM-bandwidth bound: enc must stream through SBUF once.
The host converts enc to fp16 (plus a 257th ones column per [s, b, :] row,
so one matmul per (tile, b) produces both the context contribution and the
l partial), halving HBM traffic vs f32. fp16 (not bf16: bf16's coarser
mantissa puts ~0.4% on each score element, which exp() amplifies to a
2.6e-2 end-to-end error — measured) keeps scores to ~0.02% per element;
end-to-end rel err ~3e-3. enc values are N(0,1) so fp16 range is safe.
The weights w = exp(scores-64) reach ~1e5+ and would overflow fp16, so
the w tile is bf16; the ctx matmul is mixed bf16 lhsT x fp16 rhs.

Per-core dataflow, ACTIVE config (8 tiles of [128s x (32b*258)] fp16,
~2.11 MiB each; hp=258 = 256 enc + ones col + zero pad so every per-b
slice is pair-aligned):
  - scores: 32 per-b TTR2C custom-DVE ops in the hand-authored 2X_1PORT
    perf slot (~179ns/col, 2 fp16 elems/cycle). The op is a fused
    multiply-SCAN: blk0/blk1 multiply the packed pair, blk2 sums it,
    blk3 keeps the f32 running total in its own flop, and the running
    value is written out each cycle (pair-duplicated). score_b =
    out[:, 255], gathered for all b in one strided ACT copy. The scan
    shape exists because the 2x ACCUMULATOR readout is broken in
    firmware (accum_out never written in pair mode — HW-verified);
    the out-path running sum is exact. Seed uop inits blk3's flop, so
    each instruction restarts at zero.
  - w = exp(scores - 64) on ACT (one [P,32] instr), output bf16.
  - ctx|l: per-b matmul, lhsT = w column [128,1] bf16, rhs = enc b-slice
    [128,258] fp16, PSUM(f32)-accumulated across all 8 tiles. PSUM layout:
    row 32*(b%4), bank b//4 (cols (b//4)*512..+258). One accumulation chain
    per (partition-group, bank): matmul start=True clears has_written for
    the written partitions across the whole 2KB bank, so chains must not
    share one. l rides the ones column (col 256 of each block).
Engine budget/tile: DMA ~6.3us (the floor), DVE ~5.9us, ACT ~1.5us,
PE ~3.5us -> memory-bound. Measured 51431ns vs 70628ns for the old
DVE/ACT-balanced half-mode config ({"builder": 1, "kwargs": {}}).
Host combines the per-core partials (tiny f32 arrays).
"""

from contextlib import ExitStack, nullcontext

import numpy as np
import ml_dtypes

import concourse.bacc as bacc
import concourse.bass as bass
import concourse.tile as tile
from concourse import mybir
from concourse.bass_utils import run_bass_kernel_spmd

S, B, H = 8192, 32, 256
HP = H + 1  # 257: enc columns + ones column (l accumulator)
NCORES = 8
S_CORE = S // NCORES  # 1024
P = 128
NTILES = S_CORE // P  # 8
BH = B * H  # 8192
BHP = B * HP  # 8224
EXP_SHIFT = 64.0

FP16_NP = np.float16
BF16_NP = ml_dtypes.bfloat16


def register_ttr2c():
    """Custom DVE op TTR2C: out = in0*in1, accum_out = C0 + sum(out), with a
    hand-authored 2X_1PORT uop program processing the packed fp16 pair:
      blk0: p0 = SRC_0 * SRC_1
      blk1: p1 = SRC_0_HI * SRC_1_HI   (p0 captured into delay lane 0)
      blk2: s  = p0 + p1               (p0 carried, p1 captured lane 1)
      blk3: acc += s  (CURR_ALU_OUT + PREV_ALU_OUT, alu_out_a_enable)
      blk4-7: passthrough; WR0_LO = DELAY_0 (p0), WR0_HI = DELAY_1 (p1)
    """
    import copy as _copy
    import concourse.dve_ops as dops
    from concourse.dve_uop import DveOpSpec
    from concourse.dve_spec import lower, Spec, Src0, Src1, C0
    import concourse.dve_uop as du
    from operator import add as _add
    import numpy as _np

    for o in dops.OPS:
        if o.name == "TTR2C":
            return o

    def _ref(in0, in1, c0, c1, c2):
        b = (in0.astype(_np.float32) * in1).astype(_np.float32)
        return b, c0 + b.reshape(b.shape[0], -1).sum(axis=-1, keepdims=True)

    spec = Spec(body=Src0 * Src1, accum=_add, accum_init=C0, reference=_ref)
    base = lower(spec, ver="v3")
    assert len(base) == 2, len(base)

    InpSel, AluInp, AluOp, DelayInp, OutPath, OutSel = (
        du.InpSel, du.AluInp, du.AluOp, du.DelayInp, du.OutPath, du.OutSel
    )
    u2 = [_copy.deepcopy(base[0]), _copy.deepcopy(base[1])]
    st = u2[1]  # steady-state pair uop
    # input lanes: 1<-SRC_0, 2<-SRC_1, 3<-SRC_0_HI, 4<-SRC_1_HI
    st.inp = list(st.inp)
    st.inp_enable = list(st.inp_enable)
    for lane, sel in ((1, InpSel.SRC_0), (2, InpSel.SRC_1),
                      (3, InpSel.SRC_0_HI), (4, InpSel.SRC_1_HI)):
        st.inp[lane] = sel
        st.inp_enable[lane] = 1
    dp = st.datapath_config
    carry4 = [DelayInp.PREV_DELAY] * 4 + [DelayInp.PREV_ALU_OUT] * 3
    en4 = [1, 1, 1, 1, 0, 0, 0]
    # blk0: p0 = lane0*lane1 (SRC_0*SRC_1); carry all 4 input lanes
    b = dp[0]
    b.op = AluOp.MULTIPLY
    b.alu_src0 = AluInp.PREV_DELAY_0
    b.alu_src1 = AluInp.PREV_DELAY_1
    b.delay = list(carry4)
    b.delay_enable = list(en4)
    b.alu_out_enable = 1
    b.alu_out_a_enable = 0
    # blk1: p1 = SRC_0_HI*SRC_1_HI; lane0 captures p0
    b = dp[1]
    b.op = AluOp.MULTIPLY
    b.alu_src0 = AluInp.PREV_DELAY_2
    b.alu_src1 = AluInp.PREV_DELAY_3
    b.delay = [DelayInp.PREV_ALU_OUT] + [DelayInp.PREV_DELAY] * 3 + \
        [DelayInp.PREV_ALU_OUT] * 3
    b.delay_enable = [1, 0, 0, 0, 0, 0, 0]
    b.alu_out_enable = 1
    b.alu_out_a_enable = 0
    # blk2: s = p0 + p1; lane0 carries p0, lane1 captures p1
    b = dp[2]
    b.op = AluOp.ADD
    b.alu_src0 = AluInp.PREV_DELAY_0
    b.alu_src1 = AluInp.PREV_ALU_OUT
    b.delay = [DelayInp.PREV_DELAY, DelayInp.PREV_ALU_OUT] + \
        [DelayInp.PREV_ALU_OUT] * 5
    b.delay_enable = [1, 1, 0, 0, 0, 0, 0]
    b.alu_out_enable = 1
    b.alu_out_a_enable = 0
    # blk3: acc += s
    b = dp[3]
    b.op = AluOp.ADD
    b.alu_src0 = AluInp.CURR_ALU_OUT
    b.alu_src1 = AluInp.PREV_ALU_OUT
    b.delay = [DelayInp.PREV_DELAY, DelayInp.PREV_DELAY] + \
        [DelayInp.PREV_ALU_OUT] * 5
    b.delay_enable = [1, 1, 0, 0, 0, 0, 0]
    b.alu_out_enable = 1
    b.alu_out_a_enable = 1
    # blk4-7: passthrough, keep acc flowing and p0/p1 in lanes 0/1
    for i in range(4, 8):
        b = dp[i]
        b.op = base[1].datapath_config[4].op
        b.alu_src0 = base[1].datapath_config[4].alu_src0
        b.alu_src1 = base[1].datapath_config[4].alu_src1
        b.delay = [DelayInp.PREV_DELAY, DelayInp.PREV_DELAY] + \
            [DelayInp.PREV_ALU_OUT] * 5
        b.delay_enable = [1, 1, 0, 0, 0, 0, 0]
        b.alu_out_enable = 1
        b.alu_out_a_enable = 1
    # v5 (HW-validated): out LO/HI = the blk3 running sum routed through
    # the blk4-7 ALU passthrough; seed shifted to init BLK3's flop. Each
    # out pair holds the prefix through that pair; element 255 is the
    # full 256-element dot product (f32 state, one fp16 downcast).
    st.out = dict(st.out)
    st.out_enable = dict(st.out_enable)
    st.out[OutPath.WR0_LO] = OutSel.ALU_OUT
    st.out_enable[OutPath.WR0_LO] = 1
    st.out[OutPath.WR0_HI] = OutSel.ALU_OUT
    st.out_enable[OutPath.WR0_HI] = 1
    sd = u2[0]
    sd.datapath_config[3] = _copy.deepcopy(sd.datapath_config[2])
    sd.datapath_config[2] = _copy.deepcopy(sd.datapath_config[1])

    dops._SUB_OPCODE_FOR_NAME.setdefault(
        "TTR2C_PAD", max(dops._SUB_OPCODE_FOR_NAME.values()) + 1)
    row = max(dops._SUB_OPCODE_FOR_NAME.values()) + 1
    assert row < 0x20, row
    op = dops.DveOp("TTR2C", spec, subdim=False, uops_sha={})
    dops.OPS.append(op)
    dops.CUSTOM_DVE_SPECS["TTR2C"] = spec
    dops._SUB_OPCODE_FOR_NAME["TTR2C"] = row
    ospec = DveOpSpec(
        name="TTR2C", opcode=row, uops=base, uops_2x=u2,
        rd1_en=True, perf_max=1,
    )
    ospec.validate("v3")
    dops._COMPILE_CACHE[("TTR2C", "v3")] = ospec
    return op



def register_amr2x():
    """Clone of AFFINE_MUL_REDUCE with the 2X_1PORT perf-mode table slot
    populated with the same uop program and perf_max=1 (byte-36[7:6]) so
    the DVE engine may run packed-fp16 pairs at 2 elem/cycle. The stock op
    never fills the perf slots; HW-validated numerics in microbench.py."""
    import concourse.dve_ops as dops
    from concourse.dve_uop import DveOpSpec
    from concourse.dve_spec import lower
    for o in dops.OPS:
        if o.name == "AMR_2X":
            return o
    spec = dops.AFFINE_MUL_REDUCE.spec
    row = max(dops._SUB_OPCODE_FOR_NAME.values()) + 1
    assert row < 0x20, row
    op = dops.DveOp("AMR_2X", spec, subdim=False, uops_sha={})
    dops.OPS.append(op)
    dops.CUSTOM_DVE_SPECS["AMR_2X"] = spec
    dops._SUB_OPCODE_FOR_NAME["AMR_2X"] = row
    dops._COMPILE_CACHE[("AMR_2X", "v3")] = DveOpSpec(
        name="AMR_2X", opcode=row,
        uops=lower(spec, ver="v3"),
        uops_2x=lower(spec, ver="v3"),
        rd1_en=True, perf_max=1,
    )
    return op

F32 = mybir.dt.float32
BF16 = mybir.dt.bfloat16
FP16 = mybir.dt.float16


def _build_nc(
    repeat: int = 1,
    ttr_mode: str = "alt",  # which tiles take the DVE fused path
    n_ttr: int = 4,         # tiles on DVE TTR path when ttr_mode == "first"
    mul_engine: str = "vector",  # engine for the bulk multiply on non-TTR tiles
    mul_chunk: int = 8,
    exp_group: int = 16,    # b-columns per exp instruction
    small_bufs: int = 2,
    enc_bufs: int = 3,      # enc tile pool depth (DMA lookahead)
    tmp_bufs: int = 2,      # product tile pool depth (mul -> ACT accum)
    # Winning config (measured via the hw-loop slope bench): every tile
    # splits its 32 batch columns 20 on DVE TTR / 12 on DVE-mul + ACT
    # accumulate, with the ACT-feeding muls emitted first. Both vector
    # engines land at ~8.8us/tile, just above the 6.4us/tile DMA floor.
    tile_modes: str | None = ",".join(["half"] * 8),
    half_nb: int = 20,      # b-count on the DVE TTR path in "half" tiles
    fixed_scratch: bool = False,  # single fixed scr/ascr scratch tiles
                                  # (measured: serializes WAR, much slower —
                                  # keep False)
    mul_first: bool = True,  # half mode: emit ACT-feeding muls before TTRs
                             # so ACT starts while DVE runs TTRs
    exp_align: bool = False,  # half mode: align exp groups to the
                              # half_nb split so the ACT-half exp (ready
                              # first) never waits on late TTR columns
    hw_loop: int = 0,  # bench-only: wrap the tile pipeline in a hardware
                       # loop of this count (PSUM accumulates with
                       # start=False onto the pre-zeroed tile, so the body
                       # is iteration-invariant; output values are NOT the
                       # real result — timing use only)
    ttr2x: bool = False,  # use the AMR_2X custom op (2x perf slot) for TTRs
    hp: int = HP,  # enc cols per b; 258 keeps b-slices pair-aligned for 2x
):
    nc = bacc.Bacc("TRN2", target_bir_lowering=False, debug=False)
    bhp = B * hp
    op2 = register_ttr2c() if ttr2x else None

    enc = nc.dram_tensor("enc", [S_CORE, B, hp], FP16, kind="ExternalInput")
    hidb = nc.dram_tensor("hidb", [1, BH], FP16, kind="ExternalInput")
    # 32-byte dummy input consumed by one DMA; exists so benchmarking can
    # thread a data dependency between chained executions (defeats XLA CSE)
    seed = nc.dram_tensor("seed", [1, 8], F32, kind="ExternalInput")
    ctx_raw = nc.dram_tensor("ctx_raw", [4, 4096], F32, kind="ExternalOutput")

    enc_v = enc[:].rearrange("(t p) b h -> t p (b h)", p=P)

    EXP = mybir.ActivationFunctionType.Exp
    COPY = mybir.ActivationFunctionType.Copy

    if tile_modes is not None:
        modes = tile_modes.split(",")
        assert len(modes) == NTILES and set(modes) <= {"ttr", "casc", "act", "half"}
    else:
        modes = [
            "ttr" if ((t % 2 == 0) if ttr_mode == "alt" else (t < n_ttr))
            else "act"
            for t in range(NTILES)
        ]

    with tile.TileContext(nc) as tc, ExitStack() as ctx:
        encp = ctx.enter_context(tc.tile_pool(name="encp", bufs=enc_bufs))
        tmpp = ctx.enter_context(tc.tile_pool(name="tmpp", bufs=tmp_bufs))
        scrp = ctx.enter_context(tc.tile_pool(name="scrp", bufs=small_bufs))
        smallp = ctx.enter_context(tc.tile_pool(name="smallp", bufs=small_bufs))
        singles = ctx.enter_context(tc.tile_pool(name="singles", bufs=1))
        psump = ctx.enter_context(tc.tile_pool(name="psump", bufs=1, space="PSUM"))

        # --- one-time setup ---
        # broadcast hid to all 128 partitions during DMA (step-0 partition AP;
        # reads 16KB from HBM instead of a host-replicated 2MB tensor)
        hidB = singles.tile([P, BH], FP16)
        h_ap = hidb[:]
        hid_bcast = bass.AP(
            tensor=h_ap.tensor, offset=h_ap.offset, ap=[[0, P], [1, BH]]
        )
        nc.gpsimd.dma_start(out=hidB[:], in_=hid_bcast)

        seed_sb = singles.tile([1, 8], F32)
        nc.sync.dma_start(out=seed_sb[:], in_=seed[:])

        neg_shift = singles.tile([P, 1], F32)
        nc.vector.memset(neg_shift[:], -EXP_SHIFT)

        scr_fix = None
        ascr_fix = None
        if fixed_scratch:
            scr_fix = singles.tile([P, H], FP16)
            ascr_fix = singles.tile([P, H], FP16)

        ctx_ps = psump.tile([P, 4096], F32)
        # matmuls only target rows {0,32,64,96}; zero the tile so the final
        # full-height copy reads initialized memory (and so hw_loop mode can
        # accumulate with start=False from the first matmul)
        nc.vector.memset(ctx_ps[:], 0.0)

        mul_eng = nc.vector if mul_engine == "vector" else nc.gpsimd

        loop_cm = tc.For_i(0, hw_loop) if hw_loop else nullcontext()
        with loop_cm:
            for rt in range(repeat * NTILES):
                r, t = divmod(rt, NTILES)
                enc_t = encp.tile([P, bhp], FP16, tag="enc")
                nc.sync.dma_start(out=enc_t[:], in_=enc_v[t])

                scores_t = smallp.tile([P, B], F32, tag="scores")

                mode = modes[t]
                nb = B if mode == "ttr" else (half_nb if mode == "half" else 0)

                def emit_ttrs():
                    if ttr2x:
                        # pair-scan at 2x: out = running sums (dup pairs);
                        # score_b = out[:, 255]. Gathered in one ACT copy.
                        sct = tmpp.tile([P, BH], FP16, tag="sct")
                        for b in range(nb):
                            bi = nc.vector._custom_dve(
                                op2,
                                out=sct[:, b * H:(b + 1) * H],
                                in0=enc_t[:, b * hp:b * hp + H],
                                in1=hidB[:, b * H:(b + 1) * H],
                                s0=0.0,
                                s1=0.0,
                            )
                            bi.ins.perf_max = 1
                        sv = sct[:].rearrange(
                            "p (b h) -> p b h", h=H)[:, 0:nb, H - 1:H]
                        nc.scalar.copy(
                            scores_t[:, 0:nb].rearrange(
                                "p (b o) -> p b o", o=1),
                            sv,
                        )
                        return
                    for b in range(nb):
                        scr = scr_fix if fixed_scratch else scrp.tile(
                            [P, H], FP16, tag="scr")
                        nc.vector.affine_mul_reduce(
                            out=scr[:],
                            accum_out=scores_t[:, b:b + 1],
                            in0=enc_t[:, b * hp:b * hp + H],
                            in1=hidB[:, b * H:(b + 1) * H],
                            scale=1.0,
                            bias=0.0,
                        )

                if mode in ("ttr", "half") and not (mode == "half" and mul_first):
                    emit_ttrs()
                elif mode == "casc":
                    # 3 big DVE instructions: product, segmented reduce to
                    # fp16 partials (32-wide groups; |partial| <~ 30 so fp16
                    # is safe), f32 finish. Avoids per-b instruction
                    # overhead.
                    tmp = tmpp.tile([P, BH], FP16, tag="tmp")
                    enc_view = enc_t[:].rearrange(
                        "p (b h) -> p b h", h=hp)[:, :, 0:H]
                    hid_view = hidB[:].rearrange("p (b h) -> p b h", h=H)
                    tmp_view = tmp[:].rearrange("p (b h) -> p b h", h=H)
                    nc.vector.tensor_mul(
                        tmp_view[:, :, :], enc_view[:, :, :], hid_view[:, :, :]
                    )
                    part = scrp.tile([P, B * 8], FP16, tag="part")
                    with nc.allow_low_precision(
                        reason="fp16 partials over 32 unit-normal products; "
                        "|sum| <~ 30 so rounding is ~2e-3 absolute on scores "
                        "of std 16 — immaterial next to fp16 input rounding"
                    ):
                        nc.vector.tensor_reduce(
                            out=part[:].rearrange("p (b k) -> p b k", k=8),
                            in_=tmp[:].rearrange(
                                "p (b k l) -> p (b k) l", k=8, l=32),
                            axis=mybir.AxisListType.X,
                            op=mybir.AluOpType.add,
                        )
                    nc.vector.tensor_reduce(
                        out=scores_t[:],
                        in_=part[:].rearrange("p (b k) -> p b k", k=8),
                        axis=mybir.AxisListType.X,
                        op=mybir.AluOpType.add,
                    )
                if mode in ("act", "half"):
                    # bulk multiply (chunked so ACT accums start early),
                    # segmented accumulate on ACT
                    b_lo = 0 if mode == "act" else half_nb
                    tmp = tmpp.tile([P, BH], FP16, tag="tmp")
                    enc_view = enc_t[:].rearrange(
                        "p (b h) -> p b h", h=hp)[:, :, 0:H]
                    hid_view = hidB[:].rearrange("p (b h) -> p b h", h=H)
                    tmp_view = tmp[:].rearrange("p (b h) -> p b h", h=H)
                    CH = mul_chunk
                    for b0 in range(b_lo, B, CH):
                        b1 = min(b0 + CH, B)
                        mul_eng.tensor_mul(
                            tmp_view[:, b0:b1, :],
                            enc_view[:, b0:b1, :],
                            hid_view[:, b0:b1, :],
                        )
                        for b in range(b0, min(b0 + CH, B)):
                            ascr = ascr_fix if fixed_scratch else scrp.tile(
                                [P, H], FP16, tag="ascr")
                            nc.scalar.activation(
                                out=ascr[:],
                                in_=tmp[:, b * H:(b + 1) * H],
                                func=COPY,
                                accum_out=scores_t[:, b:b + 1],
                            )
                if mode == "half" and mul_first:
                    emit_ttrs()

                w_t = smallp.tile([P, B], BF16, tag="w")
                # exp in column groups so the first matmuls can start before
                # the whole tile's scores are done; in mul_first half mode
                # the ACT-half scores finish first, so exp/matmul that half
                # first
                b_order = list(range(B))
                if mode == "half" and mul_first and exp_align:
                    groups = [(half_nb, B)] + [
                        (g, min(g + exp_group, half_nb))
                        for g in range(0, half_nb, exp_group)
                    ]
                    b_order = list(range(half_nb, B)) + list(range(half_nb))
                elif mode == "half" and mul_first:
                    g0s = list(range(0, B, exp_group))
                    g0s = [g for g in g0s if g >= half_nb] + \
                          [g for g in g0s if g < half_nb]
                    groups = [(g, min(g + exp_group, B)) for g in g0s]
                    b_order = list(range(half_nb, B)) + list(range(half_nb))
                else:
                    groups = [(g, min(g + exp_group, B))
                              for g in range(0, B, exp_group)]
                for g0, g1 in groups:
                    nc.scalar.activation(
                        out=w_t[:, g0:g1],
                        in_=scores_t[:, g0:g1],
                        func=EXP,
                        bias=neg_shift[:],
                        scale=1.0,
                    )

                first = rt == 0 and not hw_loop
                last = rt == repeat * NTILES - 1
                for b in b_order:
                    lhs = w_t[:, b:b + 1]
                    rhs = enc_t[:, b * hp:(b + 1) * hp]
                    pb = 32 * (b % 4)
                    nc.tensor.matmul(
                        ctx_ps[pb:pb + 1, (b // 4) * 512:(b // 4) * 512 + hp],
                        lhsT=lhs,
                        rhs=rhs,
                        start=first,
                        stop=last,
                        tile_position=(0, pb),
                        # 4 partition-disjoint per-b chains accumulate per
                        # bank; the sim's region-level group check is too
                        # coarse.
                        skip_group_check=True,
                    )

        # --- drain psum and store (only rows {0,32,64,96} hold results) ---
        ctx_sb = singles.tile([P, 4096], F32)
        nc.scalar.copy(ctx_sb[:], ctx_ps[:])
        for g in range(4):
            nc.sync.dma_start(
                out=ctx_raw[g:g + 1, :], in_=ctx_sb[32 * g:32 * g + 1, :]
            )

    nc.compile()
    return nc


def _build_nc2(
    repeat: int = 1,
    n_casc: int = 32,   # cols via big DVE mul + segmented reduce
    n_ttr: int = 0,     # cols via DVE fused TTR
    n_pool: int = 0,    # cols whose mul runs on Pool (gpsimd), accum on ACT
    n_act: int = 0,     # cols whose mul runs on DVE bulk, accum on ACT
    red_k: int = 8,     # stage-1 groups per col (segment width = H // red_k)
    exp_group: int = 16,
    exp_bias_float: bool = True,  # float-literal exp bias (AP bias is ~1.2us
                                  # slower per instr, measured)
    enc_bufs: int = 3,
    tmp_bufs: int = 2,
    small_bufs: int = 2,
    mul_split: int = 1,  # split the casc bulk mul into this many instrs
    hw_loop: int = 0,
):
    """Flat-layout build: enc tiles are [128, 32*256] fp16 (no ones column).
    Per-core l partial comes from a per-tile PE matmul with a ones lhsT
    (out row = PSUM partition 1, bank 0 cols 256:288 — partition-disjoint
    from the ctx rows {0,32,64,96}, so its accumulation chain is
    independent). Scores come from a configurable column split:
      casc: one big DVE fp16 mul + 32-wide segmented reduce to fp16
            partials + small f32 finish (big instrs -> 2x/4x DVE modes)
      ttr:  per-col DVE fused affine_mul_reduce
      pool: per-col Pool (gpsimd) mul, ACT copy-accumulate
      act:  DVE bulk mul, ACT copy-accumulate
    """
    assert n_casc + n_ttr + n_pool + n_act == B
    nc = bacc.Bacc("TRN2", target_bir_lowering=False, debug=False)

    enc = nc.dram_tensor("enc", [S_CORE, B, H], FP16, kind="ExternalInput")
    hidb = nc.dram_tensor("hidb", [1, BH], FP16, kind="ExternalInput")
    seed = nc.dram_tensor("seed", [1, 8], F32, kind="ExternalInput")
    ctx_raw = nc.dram_tensor("ctx_raw", [4, 4096], F32, kind="ExternalOutput")
    l_raw = nc.dram_tensor("l_raw", [1, B], F32, kind="ExternalOutput")

    enc_v = enc[:].rearrange("(t p) b h -> t p (b h)", p=P)

    EXP = mybir.ActivationFunctionType.Exp
    COPY = mybir.ActivationFunctionType.Copy

    RED_L = H // red_k  # stage-1 segment width

    with tile.TileContext(nc) as tc, ExitStack() as ctx:
        encp = ctx.enter_context(tc.tile_pool(name="encp", bufs=enc_bufs))
        tmpp = ctx.enter_context(tc.tile_pool(name="tmpp", bufs=tmp_bufs))
        scrp = ctx.enter_context(tc.tile_pool(name="scrp", bufs=small_bufs))
        smallp = ctx.enter_context(tc.tile_pool(name="smallp", bufs=small_bufs))
        singles = ctx.enter_context(tc.tile_pool(name="singles", bufs=1))
        psump = ctx.enter_context(tc.tile_pool(name="psump", bufs=1, space="PSUM"))

        hidB = singles.tile([P, BH], FP16)
        h_ap = hidb[:]
        hid_bcast = bass.AP(
            tensor=h_ap.tensor, offset=h_ap.offset, ap=[[0, P], [1, BH]]
        )
        nc.gpsimd.dma_start(out=hidB[:], in_=hid_bcast)

        seed_sb = singles.tile([1, 8], F32)
        nc.sync.dma_start(out=seed_sb[:], in_=seed[:])

        neg_shift = singles.tile([P, 1], F32)
        nc.vector.memset(neg_shift[:], -EXP_SHIFT)
        ones_col = singles.tile([P, 1], BF16)
        nc.vector.memset(ones_col[:], 1.0)

        ctx_ps = psump.tile([P, 4096], F32)
        nc.vector.memset(ctx_ps[:], 0.0)

        loop_cm = tc.For_i(0, hw_loop) if hw_loop else nullcontext()
        with loop_cm:
            for rt in range(repeat * NTILES):
                r, t = divmod(rt, NTILES)
                enc_t = encp.tile([P, BH], FP16, tag="enc")
                nc.sync.dma_start(out=enc_t[:], in_=enc_v[t])

                scores_t = smallp.tile([P, B], F32, tag="scores")

                # --- pool-mul cols first (Pool starts early) ---
                pool_scr = []
                for b in range(n_casc + n_ttr, n_casc + n_ttr + n_pool):
                    pscr = scrp.tile([P, H], FP16, tag="pscr")
                    nc.gpsimd.tensor_mul(
                        pscr[:],
                        enc_t[:, b * H:(b + 1) * H],
                        hidB[:, b * H:(b + 1) * H],
                    )
                    pool_scr.append((b, pscr))

                # --- act cols: DVE bulk mul first (ACT starts earliest) ---
                act_lo = n_casc + n_ttr + n_pool
                if n_act:
                    atmp = tmpp.tile([P, n_act * H], FP16, tag="atmp")
                    nc.vector.tensor_mul(
                        atmp[:],
                        enc_t[:, act_lo * H:(act_lo + n_act) * H],
                        hidB[:, act_lo * H:(act_lo + n_act) * H],
                    )

                # --- casc cols: big mul + segmented reduce ---
                if n_casc:
                    nb = n_casc
                    tmp = tmpp.tile([P, nb * H], FP16, tag="tmp")
                    cw = nb * H // mul_split
                    for m in range(mul_split):
                        nc.vector.tensor_mul(
                            tmp[:, m * cw:(m + 1) * cw],
                            enc_t[:, m * cw:(m + 1) * cw],
                            hidB[:, m * cw:(m + 1) * cw],
                        )
                    part = scrp.tile([P, nb * red_k], FP16, tag="part")
                    with nc.allow_low_precision(
                        reason="fp16 partials over <=32-wide unit-normal "
                        "product groups; |partial| <~ 30, rounding ~2e-3 "
                        "absolute on scores of std 16"
                    ):
                        nc.vector.tensor_reduce(
                            out=part[:].rearrange(
                                "p (b k) -> p b k", k=red_k),
                            in_=tmp[:].rearrange(
                                "p (b k l) -> p (b k) l", k=red_k, l=RED_L),
                            axis=mybir.AxisListType.X,
                            op=mybir.AluOpType.add,
                        )
                    nc.vector.tensor_reduce(
                        out=scores_t[:, 0:nb],
                        in_=part[:].rearrange("p (b k) -> p b k", k=red_k),
                        axis=mybir.AxisListType.X,
                        op=mybir.AluOpType.add,
                    )

                # --- ttr cols ---
                for b in range(n_casc, n_casc + n_ttr):
                    scr = scrp.tile([P, H], FP16, tag="scr")
                    nc.vector.affine_mul_reduce(
                        out=scr[:],
                        accum_out=scores_t[:, b:b + 1],
                        in0=enc_t[:, b * H:(b + 1) * H],
                        in1=hidB[:, b * H:(b + 1) * H],
                        scale=1.0,
                        bias=0.0,
                    )

                # --- ACT accumulates for pool/act cols ---
                for b, pscr in pool_scr:
                    ascr = scrp.tile([P, H], FP16, tag="ascr")
                    nc.scalar.activation(
                        out=ascr[:],
                        in_=pscr[:],
                        func=COPY,
                        accum_out=scores_t[:, b:b + 1],
                    )
                for j in range(n_act):
                    b = act_lo + j
                    ascr = scrp.tile([P, H], FP16, tag="ascr")
                    nc.scalar.activation(
                        out=ascr[:],
                        in_=atmp[:, j * H:(j + 1) * H],
                        func=COPY,
                        accum_out=scores_t[:, b:b + 1],
                    )

                # --- exp ---
                w_t = smallp.tile([P, B], BF16, tag="w")
                for g0 in range(0, B, exp_group):
                    g1 = min(g0 + exp_group, B)
                    nc.scalar.activation(
                        out=w_t[:, g0:g1],
                        in_=scores_t[:, g0:g1],
                        func=EXP,
                        bias=-EXP_SHIFT if exp_bias_float else neg_shift[:],
                        scale=1.0,
                    )

                # --- ctx matmuls + l matmul ---
                first = rt == 0 and not hw_loop
                last = rt == repeat * NTILES - 1
                for b in range(B):
                    pb = 32 * (b % 4)
                    nc.tensor.matmul(
                        ctx_ps[pb:pb + 1, (b // 4) * 512:(b // 4) * 512 + H],
                        lhsT=w_t[:, b:b + 1],
                        rhs=enc_t[:, b * H:(b + 1) * H],
                        start=first,
                        stop=last,
                        tile_position=(0, pb),
                        skip_group_check=True,
                    )
                nc.tensor.matmul(
                    ctx_ps[1:2, 256:256 + B],
                    lhsT=ones_col[:],
                    rhs=w_t[:],
                    start=first,
                    stop=last,
                    tile_position=(0, 0),
                    skip_group_check=True,
                )

        # drain PSUM -> SBUF (ACT copy; DMA cannot read PSUM) -> HBM
        ctx_sb = singles.tile([P, 4096], F32)
        nc.scalar.copy(ctx_sb[0:97, :], ctx_ps[0:97, :])
        for g in range(4):
            nc.sync.dma_start(
                out=ctx_raw[g:g + 1, :], in_=ctx_sb[32 * g:32 * g + 1, :]
            )
        nc.sync.dma_start(out=l_raw[:], in_=ctx_sb[1:2, 256:256 + B])

    nc.compile()
    return nc


# Active configuration: builder + kwargs used by kernel() and the timing
# harness (test.py --time calls _build_current).
# Builder 1 with ttr2x: all 32 score columns via the TTR2C custom DVE op
# running in the hand-authored 2X_1PORT slot (~179ns/col vs 313 at 1x),
# scores read as the scan's last element (the 2x accumulator readout is
# broken in firmware; the running-sum output path is HW-validated exact).
# hp=258 keeps every operand pair-aligned so the engine never falls back
# to 1x. Fallback proven config: {"builder": 1, "kwargs": {}} (~70.6us).
_CONFIG = {
    "builder": 1,
    "kwargs": dict(
        tile_modes=",".join(["ttr"] * 8), ttr2x=True, hp=258,
        enc_bufs=4, exp_group=32,
    ),
}


def _build_current(**extra):
    kw = dict(_CONFIG["kwargs"])
    kw.update(extra)
    if _CONFIG["builder"] == 2:
        return _build_nc2(**kw)
    return _build_nc(**kw)


_NC_CACHE = {}


def _get_nc():
    if "nc" not in _NC_CACHE:
        _NC_CACHE["nc"] = _build_current()
    return _NC_CACHE["nc"]


def _augment_enc(enc_slice: np.ndarray, hp: int = HP) -> np.ndarray:
    """[S_CORE, B, H] f32 -> [S_CORE, B, hp] fp16: ones col at H, zero pad."""
    out = np.zeros((S_CORE, B, hp), dtype=FP16_NP)
    out[:, :, :H] = enc_slice
    out[:, :, H] = 1.0
    return out


def _prep_enc(enc_slice: np.ndarray) -> np.ndarray:
    """Per-core enc input for the active builder."""
    if _CONFIG["builder"] == 2:
        return np.ascontiguousarray(enc_slice, dtype=FP16_NP)
    return _augment_enc(enc_slice, _CONFIG["kwargs"].get("hp", HP))


def kernel(enc_output_i: np.ndarray, enc_or_dec_hid_i: np.ndarray) -> np.ndarray:
    enc = np.asarray(enc_output_i, dtype=np.float32)
    hid = np.asarray(enc_or_dec_hid_i, dtype=np.float32)[0]  # [B, H]

    hidb = np.ascontiguousarray(hid.reshape(1, BH)).astype(FP16_NP)

    nc = _get_nc()
    zseed = np.zeros((1, 8), dtype=np.float32)
    in_maps = [
        {
            "enc": _prep_enc(enc[c * S_CORE:(c + 1) * S_CORE]),
            "hidb": hidb,
            "seed": zseed,
        }
        for c in range(NCORES)
    ]
    results = run_bass_kernel_spmd(nc, in_maps, core_ids=list(range(NCORES))).results

    ctx_sum = np.zeros((B, H), dtype=np.float64)
    l_sum = np.zeros((B,), dtype=np.float64)
    for c in range(NCORES):
        raw = results[c]["ctx_raw"]  # [4, 4096]; row = b%4, col block b//4
        g = raw.reshape(4, 8, 512)
        g = np.transpose(g, (1, 0, 2)).reshape(B, 512)  # [b, 512]
        ctx_sum += g[:, :H]
        if _CONFIG["builder"] == 2:
            l_sum += results[c]["l_raw"][0]
        else:
            l_sum += g[:, H]
    out = (ctx_sum / l_sum[:, None]).astype(np.float32)
    return out



# revision 31
# speedup vs baseline: 1.2113x; 1.2113x over previous
"""Attention-pooling kernel for TRN2 (8 NeuronCores, SPMD).

Problem: enc [S=8192, B=32, H=256] f32, hid [1, B, H] f32.
  scores = einsum('sbh,bh->bs'); w = softmax(scores, axis=s)
  ctx    = einsum('sbh,bs->bh')

Two builders are kept:
  _build_nc  — the original ones-column layout (enc cols = 257 per b; l
               rides the ctx matmul).
  _build_nc2 — flat [128, 32*256] fp16 tiles, l via a per-tile PE matmul
               with a ones lhsT into PSUM partition 1 (partition-disjoint
               chain), single ACT drain copy of PSUM rows 0..96.
               Score engine split is parameterized (n_ttr on DVE fused
               affine_mul_reduce at ~313ns/col, n_act on DVE-bulk-mul@2x
               + ACT copy-accumulate at ~134+~650ns/col); measured-
               balanced split is 21/11. Microbenches (microbench.py) show
               DVE fp16 tensor_mul runs in 2x mode (0.51ns/elem) but
               tensor_reduce and custom-DVE ops are 1x, Pool serializes
               with DVE (shared SBUF port), and per-core DMA caps at
               ~337GB/s regardless of queue count — so the kernel floor
               is the DVE/ACT score balance (~8.1us/tile), slightly above
               the 6.3us/tile DMA floor.

Sharding: S split into 8 contiguous 1024-row slices (one per core); softmax
is decomposed as per-core partial sums with a *fixed* exponent shift C:
  w_c = exp(scores_c - C);  l_c = sum_s w_c;  ctx_c = sum_s w_c * enc
  ctx = sum_c ctx_c / sum_c l_c
The shift C=64 keeps exp in f32 range for this problem's score magnitudes
(max |score| ~ 76; exp(76-64)=e^12 ~ 1.6e5, far below f32 max) and cancels
exactly in the final division, so no cross-core max pass is needed.

The problem is HBM-bandwidth bound: enc must stream through SBUF once.
The host converts enc to fp16 (plus a 257th ones column per [s, b, :] row,
so one matmul per (tile, b) produces both the context contribution and the
l partial), halving HBM traffic vs f32. fp16 (not bf16: bf16's coarser
mantissa puts ~0.4% on each score element, which exp() amplifies to a
2.6e-2 end-to-end error — measured) keeps scores to ~0.02% per element;
end-to-end rel err ~3e-3. enc values are N(0,1) so fp16 range is safe.
The weights w = exp(scores-64) reach ~1e5+ and would overflow fp16, so
the w tile is bf16; the ctx matmul is mixed bf16 lhsT x fp16 rhs.

Per-core dataflow, ACTIVE config (8 tiles of [128s x (32b*258)] fp16,
~2.11 MiB each; hp=258 = 256 enc + ones col + zero pad so every per-b
slice is pair-aligned):
  - scores: 32 per-b TTR2C custom-DVE ops in the hand-authored 2X_1PORT
    perf slot (~179ns/col, 2 fp16 elems/cycle). The op is a fused
    multiply-SCAN: blk0/blk1 multiply the packed pair, blk2 sums it,
    blk3 keeps the f32 running total in its own flop, and the running
    value is written out each cycle (pair-duplicated). score_b =
    out[:, 255], gathered for all b in one strided ACT copy. The scan
    shape exists because the 2x ACCUMULATOR readout is broken in
    firmware (accum_out never written in pair mode — HW-verified);
    the out-path running sum is exact. Seed uop inits blk3's flop, so
    each instruction restarts at zero.
  - w = exp(scores - 64) on ACT (two [P,16] instrs), output bf16.
  - ctx|l: per-b matmul, lhsT = w column [128,1] bf16, rhs = enc b-slice
    [128,258] fp16, PSUM(f32)-accumulated across all 8 tiles. PSUM layout:
    row 32*(b%4), bank b//4 (cols (b//4)*512..+258). One accumulation chain
    per (partition-group, bank): matmul start=True clears has_written for
    the written partitions across the whole 2KB bank, so chains must not
    share one. l rides the ones column (col 256 of each block).
Engine budget/tile: DMA ~6.3us (the floor), DVE ~5.9us, ACT ~1.5us,
PE ~3.5us -> memory-bound. Measured 51431ns vs 70628ns for the old
DVE/ACT-balanced half-mode config ({"builder": 1, "kwargs": {}}).
Host combines the per-core partials (tiny f32 arrays).
"""

from contextlib import ExitStack, nullcontext

import numpy as np
import ml_dtypes

import concourse.bacc as bacc
import concourse.bass as bass
import concourse.tile as tile
from concourse import mybir
from concourse.bass_utils import run_bass_kernel_spmd

S, B, H = 8192, 32, 256
HP = H + 1  # 257: enc columns + ones column (l accumulator)
NCORES = 8
S_CORE = S // NCORES  # 1024
P = 128
NTILES = S_CORE // P  # 8
BH = B * H  # 8192
BHP = B * HP  # 8224
EXP_SHIFT = 64.0

FP16_NP = np.float16
BF16_NP = ml_dtypes.bfloat16


def register_ttr2c():
    """Custom DVE op TTR2C: out = in0*in1, accum_out = C0 + sum(out), with a
    hand-authored 2X_1PORT uop program processing the packed fp16 pair:
      blk0: p0 = SRC_0 * SRC_1
      blk1: p1 = SRC_0_HI * SRC_1_HI   (p0 captured into delay lane 0)
      blk2: s  = p0 + p1               (p0 carried, p1 captured lane 1)
      blk3: acc += s  (CURR_ALU_OUT + PREV_ALU_OUT, alu_out_a_enable)
      blk4-7: passthrough; WR0_LO = DELAY_0 (p0), WR0_HI = DELAY_1 (p1)
    """
    import copy as _copy
    import concourse.dve_ops as dops
    from concourse.dve_uop import DveOpSpec
    from concourse.dve_spec import lower, Spec, Src0, Src1, C0
    import concourse.dve_uop as du
    from operator import add as _add
    import numpy as _np

    for o in dops.OPS:
        if o.name == "TTR2C":
            return o

    def _ref(in0, in1, c0, c1, c2):
        b = (in0.astype(_np.float32) * in1).astype(_np.float32)
        return b, c0 + b.reshape(b.shape[0], -1).sum(axis=-1, keepdims=True)

    spec = Spec(body=Src0 * Src1, accum=_add, accum_init=C0, reference=_ref)
    base = lower(spec, ver="v3")
    assert len(base) == 2, len(base)

    InpSel, AluInp, AluOp, DelayInp, OutPath, OutSel = (
        du.InpSel, du.AluInp, du.AluOp, du.DelayInp, du.OutPath, du.OutSel
    )
    u2 = [_copy.deepcopy(base[0]), _copy.deepcopy(base[1])]
    st = u2[1]  # steady-state pair uop
    # input lanes: 1<-SRC_0, 2<-SRC_1, 3<-SRC_0_HI, 4<-SRC_1_HI
    st.inp = list(st.inp)
    st.inp_enable = list(st.inp_enable)
    for lane, sel in ((1, InpSel.SRC_0), (2, InpSel.SRC_1),
                      (3, InpSel.SRC_0_HI), (4, InpSel.SRC_1_HI)):
        st.inp[lane] = sel
        st.inp_enable[lane] = 1
    dp = st.datapath_config
    carry4 = [DelayInp.PREV_DELAY] * 4 + [DelayInp.PREV_ALU_OUT] * 3
    en4 = [1, 1, 1, 1, 0, 0, 0]
    # blk0: p0 = lane0*lane1 (SRC_0*SRC_1); carry all 4 input lanes
    b = dp[0]
    b.op = AluOp.MULTIPLY
    b.alu_src0 = AluInp.PREV_DELAY_0
    b.alu_src1 = AluInp.PREV_DELAY_1
    b.delay = list(carry4)
    b.delay_enable = list(en4)
    b.alu_out_enable = 1
    b.alu_out_a_enable = 0
    # blk1: p1 = SRC_0_HI*SRC_1_HI; lane0 captures p0
    b = dp[1]
    b.op = AluOp.MULTIPLY
    b.alu_src0 = AluInp.PREV_DELAY_2
    b.alu_src1 = AluInp.PREV_DELAY_3
    b.delay = [DelayInp.PREV_ALU_OUT] + [DelayInp.PREV_DELAY] * 3 + \
        [DelayInp.PREV_ALU_OUT] * 3
    b.delay_enable = [1, 0, 0, 0, 0, 0, 0]
    b.alu_out_enable = 1
    b.alu_out_a_enable = 0
    # blk2: s = p0 + p1; lane0 carries p0, lane1 captures p1
    b = dp[2]
    b.op = AluOp.ADD
    b.alu_src0 = AluInp.PREV_DELAY_0
    b.alu_src1 = AluInp.PREV_ALU_OUT
    b.delay = [DelayInp.PREV_DELAY, DelayInp.PREV_ALU_OUT] + \
        [DelayInp.PREV_ALU_OUT] * 5
    b.delay_enable = [1, 1, 0, 0, 0, 0, 0]
    b.alu_out_enable = 1
    b.alu_out_a_enable = 0
    # blk3: acc += s
    b = dp[3]
    b.op = AluOp.ADD
    b.alu_src0 = AluInp.CURR_ALU_OUT
    b.alu_src1 = AluInp.PREV_ALU_OUT
    b.delay = [DelayInp.PREV_DELAY, DelayInp.PREV_DELAY] + \
        [DelayInp.PREV_ALU_OUT] * 5
    b.delay_enable = [1, 1, 0, 0, 0, 0, 0]
    b.alu_out_enable = 1
    b.alu_out_a_enable = 1
    # blk4-7: passthrough, keep acc flowing and p0/p1 in lanes 0/1
    for i in range(4, 8):
        b = dp[i]
        b.op = base[1].datapath_config[4].op
        b.alu_src0 = base[1].datapath_config[4].alu_src0
        b.alu_src1 = base[1].datapath_config[4].alu_src1
        b.delay = [DelayInp.PREV_DELAY, DelayInp.PREV_DELAY] + \
            [DelayInp.PREV_ALU_OUT] * 5
        b.delay_enable = [1, 1, 0, 0, 0, 0, 0]
        b.alu_out_enable = 1
        b.alu_out_a_enable = 1
    # v5 (HW-validated): out LO/HI = the blk3 running sum routed through
    # the blk4-7 ALU passthrough; seed shifted to init BLK3's flop. Each
    # out pair holds the prefix through that pair; element 255 is the
    # full 256-element dot product (f32 state, one fp16 downcast).
    st.out = dict(st.out)
    st.out_enable = dict(st.out_enable)
    st.out[OutPath.WR0_LO] = OutSel.ALU_OUT
    st.out_enable[OutPath.WR0_LO] = 1
    st.out[OutPath.WR0_HI] = OutSel.ALU_OUT
    st.out_enable[OutPath.WR0_HI] = 1
    sd = u2[0]
    sd.datapath_config[3] = _copy.deepcopy(sd.datapath_config[2])
    sd.datapath_config[2] = _copy.deepcopy(sd.datapath_config[1])

    dops._SUB_OPCODE_FOR_NAME.setdefault(
        "TTR2C_PAD", max(dops._SUB_OPCODE_FOR_NAME.values()) + 1)
    row = max(dops._SUB_OPCODE_FOR_NAME.values()) + 1
    assert row < 0x20, row
    op = dops.DveOp("TTR2C", spec, subdim=False, uops_sha={})
    dops.OPS.append(op)
    dops.CUSTOM_DVE_SPECS["TTR2C"] = spec
    dops._SUB_OPCODE_FOR_NAME["TTR2C"] = row
    ospec = DveOpSpec(
        name="TTR2C", opcode=row, uops=base, uops_2x=u2,
        rd1_en=True, perf_max=1,
    )
    ospec.validate("v3")
    dops._COMPILE_CACHE[("TTR2C", "v3")] = ospec
    return op



def register_amr2x():
    """Clone of AFFINE_MUL_REDUCE with the 2X_1PORT perf-mode table slot
    populated with the same uop program and perf_max=1 (byte-36[7:6]) so
    the DVE engine may run packed-fp16 pairs at 2 elem/cycle. The stock op
    never fills the perf slots; HW-validated numerics in microbench.py."""
    import concourse.dve_ops as dops
    from concourse.dve_uop import DveOpSpec
    from concourse.dve_spec import lower
    for o in dops.OPS:
        if o.name == "AMR_2X":
            return o
    spec = dops.AFFINE_MUL_REDUCE.spec
    row = max(dops._SUB_OPCODE_FOR_NAME.values()) + 1
    assert row < 0x20, row
    op = dops.DveOp("AMR_2X", spec, subdim=False, uops_sha={})
    dops.OPS.append(op)
    dops.CUSTOM_DVE_SPECS["AMR_2X"] = spec
    dops._SUB_OPCODE_FOR_NAME["AMR_2X"] = row
    dops._COMPILE_CACHE[("AMR_2X", "v3")] = DveOpSpec(
        name="AMR_2X", opcode=row,
        uops=lower(spec, ver="v3"),
        uops_2x=lower(spec, ver="v3"),
        rd1_en=True, perf_max=1,
    )
    return op

F32 = mybir.dt.float32
BF16 = mybir.dt.bfloat16
FP16 = mybir.dt.float16


def _build_nc(
    repeat: int = 1,
    ttr_mode: str = "alt",  # which tiles take the DVE fused path
    n_ttr: int = 4,         # tiles on DVE TTR path when ttr_mode == "first"
    mul_engine: str = "vector",  # engine for the bulk multiply on non-TTR tiles
    mul_chunk: int = 8,
    exp_group: int = 16,    # b-columns per exp instruction
    small_bufs: int = 2,
    enc_bufs: int = 3,      # enc tile pool depth (DMA lookahead)
    tmp_bufs: int = 2,      # product tile pool depth (mul -> ACT accum)
    # Winning config (measured via the hw-loop slope bench): every tile
    # splits its 32 batch columns 20 on DVE TTR / 12 on DVE-mul + ACT
    # accumulate, with the ACT-feeding muls emitted first. Both vector
    # engines land at ~8.8us/tile, just above the 6.4us/tile DMA floor.
    tile_modes: str | None = ",".join(["half"] * 8),
    half_nb: int = 20,      # b-count on the DVE TTR path in "half" tiles
    fixed_scratch: bool = False,  # single fixed scr/ascr scratch tiles
                                  # (measured: serializes WAR, much slower —
                                  # keep False)
    mul_first: bool = True,  # half mode: emit ACT-feeding muls before TTRs
                             # so ACT starts while DVE runs TTRs
    exp_align: bool = False,  # half mode: align exp groups to the
                              # half_nb split so the ACT-half exp (ready
                              # first) never waits on late TTR columns
    hw_loop: int = 0,  # bench-only: wrap the tile pipeline in a hardware
                       # loop of this count (PSUM accumulates with
                       # start=False onto the pre-zeroed tile, so the body
                       # is iteration-invariant; output values are NOT the
                       # real result — timing use only)
    ttr2x: bool = False,  # use the AMR_2X custom op (2x perf slot) for TTRs
    hp: int = HP,  # enc cols per b; 258 keeps b-slices pair-aligned for 2x
):
    nc = bacc.Bacc("TRN2", target_bir_lowering=False, debug=False)
    bhp = B * hp
    op2 = register_ttr2c() if ttr2x else None

    enc = nc.dram_tensor("enc", [S_CORE, B, hp], FP16, kind="ExternalInput")
    hidb = nc.dram_tensor("hidb", [1, BH], FP16, kind="ExternalInput")
    # 32-byte dummy input consumed by one DMA; exists so benchmarking can
    # thread a data dependency between chained executions (defeats XLA CSE)
    seed = nc.dram_tensor("seed", [1, 8], F32, kind="ExternalInput")
    ctx_raw = nc.dram_tensor("ctx_raw", [4, 4096], F32, kind="ExternalOutput")

    enc_v = enc[:].rearrange("(t p) b h -> t p (b h)", p=P)

    EXP = mybir.ActivationFunctionType.Exp
    COPY = mybir.ActivationFunctionType.Copy

    if tile_modes is not None:
        modes = tile_modes.split(",")
        assert len(modes) == NTILES and set(modes) <= {"ttr", "casc", "act", "half"}
    else:
        modes = [
            "ttr" if ((t % 2 == 0) if ttr_mode == "alt" else (t < n_ttr))
            else "act"
            for t in range(NTILES)
        ]

    with tile.TileContext(nc) as tc, ExitStack() as ctx:
        encp = ctx.enter_context(tc.tile_pool(name="encp", bufs=enc_bufs))
        tmpp = ctx.enter_context(tc.tile_pool(name="tmpp", bufs=tmp_bufs))
        scrp = ctx.enter_context(tc.tile_pool(name="scrp", bufs=small_bufs))
        smallp = ctx.enter_context(tc.tile_pool(name="smallp", bufs=small_bufs))
        singles = ctx.enter_context(tc.tile_pool(name="singles", bufs=1))
        psump = ctx.enter_context(tc.tile_pool(name="psump", bufs=1, space="PSUM"))

        # --- one-time setup ---
        # broadcast hid to all 128 partitions during DMA (step-0 partition AP;
        # reads 16KB from HBM instead of a host-replicated 2MB tensor)
        hidB = singles.tile([P, BH], FP16)
        h_ap = hidb[:]
        hid_bcast = bass.AP(
            tensor=h_ap.tensor, offset=h_ap.offset, ap=[[0, P], [1, BH]]
        )
        nc.gpsimd.dma_start(out=hidB[:], in_=hid_bcast)

        seed_sb = singles.tile([1, 8], F32)
        nc.sync.dma_start(out=seed_sb[:], in_=seed[:])

        neg_shift = singles.tile([P, 1], F32)
        nc.vector.memset(neg_shift[:], -EXP_SHIFT)

        scr_fix = None
        ascr_fix = None
        if fixed_scratch:
            scr_fix = singles.tile([P, H], FP16)
            ascr_fix = singles.tile([P, H], FP16)

        ctx_ps = psump.tile([P, 4096], F32)
        # matmuls only target rows {0,32,64,96}; zero the tile so the final
        # full-height copy reads initialized memory (and so hw_loop mode can
        # accumulate with start=False from the first matmul)
        nc.vector.memset(ctx_ps[:], 0.0)

        mul_eng = nc.vector if mul_engine == "vector" else nc.gpsimd

        loop_cm = tc.For_i(0, hw_loop) if hw_loop else nullcontext()
        with loop_cm:
            for rt in range(repeat * NTILES):
                r, t = divmod(rt, NTILES)
                enc_t = encp.tile([P, bhp], FP16, tag="enc")
                nc.sync.dma_start(out=enc_t[:], in_=enc_v[t])

                scores_t = smallp.tile([P, B], F32, tag="scores")

                mode = modes[t]
                nb = B if mode == "ttr" else (half_nb if mode == "half" else 0)

                def emit_ttrs():
                    if ttr2x:
                        # pair-scan at 2x: out = running sums (dup pairs);
                        # score_b = out[:, 255]. Gathered in one ACT copy.
                        sct = tmpp.tile([P, BH], FP16, tag="sct")
                        for b in range(nb):
                            bi = nc.vector._custom_dve(
                                op2,
                                out=sct[:, b * H:(b + 1) * H],
                                in0=enc_t[:, b * hp:b * hp + H],
                                in1=hidB[:, b * H:(b + 1) * H],
                                s0=0.0,
                                s1=0.0,
                            )
                            bi.ins.perf_max = 1
                        sv = sct[:].rearrange(
                            "p (b h) -> p b h", h=H)[:, 0:nb, H - 1:H]
                        nc.scalar.copy(
                            scores_t[:, 0:nb].rearrange(
                                "p (b o) -> p b o", o=1),
                            sv,
                        )
                        return
                    for b in range(nb):
                        scr = scr_fix if fixed_scratch else scrp.tile(
                            [P, H], FP16, tag="scr")
                        nc.vector.affine_mul_reduce(
                            out=scr[:],
                            accum_out=scores_t[:, b:b + 1],
                            in0=enc_t[:, b * hp:b * hp + H],
                            in1=hidB[:, b * H:(b + 1) * H],
                            scale=1.0,
                            bias=0.0,
                        )

                if mode in ("ttr", "half") and not (mode == "half" and mul_first):
                    emit_ttrs()
                elif mode == "casc":
                    # 3 big DVE instructions: product, segmented reduce to
                    # fp16 partials (32-wide groups; |partial| <~ 30 so fp16
                    # is safe), f32 finish. Avoids per-b instruction
                    # overhead.
                    tmp = tmpp.tile([P, BH], FP16, tag="tmp")
                    enc_view = enc_t[:].rearrange(
                        "p (b h) -> p b h", h=hp)[:, :, 0:H]
                    hid_view = hidB[:].rearrange("p (b h) -> p b h", h=H)
                    tmp_view = tmp[:].rearrange("p (b h) -> p b h", h=H)
                    nc.vector.tensor_mul(
                        tmp_view[:, :, :], enc_view[:, :, :], hid_view[:, :, :]
                    )
                    part = scrp.tile([P, B * 8], FP16, tag="part")
                    with nc.allow_low_precision(
                        reason="fp16 partials over 32 unit-normal products; "
                        "|sum| <~ 30 so rounding is ~2e-3 absolute on scores "
                        "of std 16 — immaterial next to fp16 input rounding"
                    ):
                        nc.vector.tensor_reduce(
                            out=part[:].rearrange("p (b k) -> p b k", k=8),
                            in_=tmp[:].rearrange(
                                "p (b k l) -> p (b k) l", k=8, l=32),
                            axis=mybir.AxisListType.X,
                            op=mybir.AluOpType.add,
                        )
                    nc.vector.tensor_reduce(
                        out=scores_t[:],
                        in_=part[:].rearrange("p (b k) -> p b k", k=8),
                        axis=mybir.AxisListType.X,
                        op=mybir.AluOpType.add,
                    )
                if mode in ("act", "half"):
                    # bulk multiply (chunked so ACT accums start early),
                    # segmented accumulate on ACT
                    b_lo = 0 if mode == "act" else half_nb
                    tmp = tmpp.tile([P, BH], FP16, tag="tmp")
                    enc_view = enc_t[:].rearrange(
                        "p (b h) -> p b h", h=hp)[:, :, 0:H]
                    hid_view = hidB[:].rearrange("p (b h) -> p b h", h=H)
                    tmp_view = tmp[:].rearrange("p (b h) -> p b h", h=H)
                    CH = mul_chunk
                    for b0 in range(b_lo, B, CH):
                        b1 = min(b0 + CH, B)
                        mul_eng.tensor_mul(
                            tmp_view[:, b0:b1, :],
                            enc_view[:, b0:b1, :],
                            hid_view[:, b0:b1, :],
                        )
                        for b in range(b0, min(b0 + CH, B)):
                            ascr = ascr_fix if fixed_scratch else scrp.tile(
                                [P, H], FP16, tag="ascr")
                            nc.scalar.activation(
                                out=ascr[:],
                                in_=tmp[:, b * H:(b + 1) * H],
                                func=COPY,
                                accum_out=scores_t[:, b:b + 1],
                            )
                if mode == "half" and mul_first:
                    emit_ttrs()

                w_t = smallp.tile([P, B], BF16, tag="w")
                # exp in column groups so the first matmuls can start before
                # the whole tile's scores are done; in mul_first half mode
                # the ACT-half scores finish first, so exp/matmul that half
                # first
                b_order = list(range(B))
                if mode == "half" and mul_first and exp_align:
                    groups = [(half_nb, B)] + [
                        (g, min(g + exp_group, half_nb))
                        for g in range(0, half_nb, exp_group)
                    ]
                    b_order = list(range(half_nb, B)) + list(range(half_nb))
                elif mode == "half" and mul_first:
                    g0s = list(range(0, B, exp_group))
                    g0s = [g for g in g0s if g >= half_nb] + \
                          [g for g in g0s if g < half_nb]
                    groups = [(g, min(g + exp_group, B)) for g in g0s]
                    b_order = list(range(half_nb, B)) + list(range(half_nb))
                else:
                    groups = [(g, min(g + exp_group, B))
                              for g in range(0, B, exp_group)]
                for g0, g1 in groups:
                    nc.scalar.activation(
                        out=w_t[:, g0:g1],
                        in_=scores_t[:, g0:g1],
                        func=EXP,
                        bias=neg_shift[:],
                        scale=1.0,
                    )

                first = rt == 0 and not hw_loop
                last = rt == repeat * NTILES - 1
                for b in b_order:
                    lhs = w_t[:, b:b + 1]
                    rhs = enc_t[:, b * hp:(b + 1) * hp]
                    pb = 32 * (b % 4)
                    nc.tensor.matmul(
                        ctx_ps[pb:pb + 1, (b // 4) * 512:(b // 4) * 512 + hp],
                        lhsT=lhs,
                        rhs=rhs,
                        start=first,
                        stop=last,
                        tile_position=(0, pb),
                        # 4 partition-disjoint per-b chains accumulate per
                        # bank; the sim's region-level group check is too
                        # coarse.
                        skip_group_check=True,
                    )

        # --- drain psum and store (only rows {0,32,64,96} hold results) ---
        ctx_sb = singles.tile([P, 4096], F32)
        nc.scalar.copy(ctx_sb[:], ctx_ps[:])
        for g in range(4):
            nc.sync.dma_start(
                out=ctx_raw[g:g + 1, :], in_=ctx_sb[32 * g:32 * g + 1, :]
            )

    nc.compile()
    return nc


def _build_nc2(
    repeat: int = 1,
    n_casc: int = 32,   # cols via big DVE mul + segmented reduce
    n_ttr: int = 0,     # cols via DVE fused TTR
    n_pool: int = 0,    # cols whose mul runs on Pool (gpsimd), accum on ACT
    n_act: int = 0,     # cols whose mul runs on DVE bulk, accum on ACT
    red_k: int = 8,     # stage-1 groups per col (segment width = H // red_k)
    exp_group: int = 16,
    exp_bias_float: bool = True,  # float-literal exp bias (AP bias is ~1.2us
                                  # slower per instr, measured)
    enc_bufs: int = 3,
    tmp_bufs: int = 2,
    small_bufs: int = 2,
    mul_split: int = 1,  # split the casc bulk mul into this many instrs
    hw_loop: int = 0,
):
    """Flat-layout build: enc tiles are [128, 32*256] fp16 (no ones column).
    Per-core l partial comes from a per-tile PE matmul with a ones lhsT
    (out row = PSUM partition 1, bank 0 cols 256:288 — partition-disjoint
    from the ctx rows {0,32,64,96}, so its accumulation chain is
    independent). Scores come from a configurable column split:
      casc: one big DVE fp16 mul + 32-wide segmented reduce to fp16
            partials + small f32 finish (big instrs -> 2x/4x DVE modes)
      ttr:  per-col DVE fused affine_mul_reduce
      pool: per-col Pool (gpsimd) mul, ACT copy-accumulate
      act:  DVE bulk mul, ACT copy-accumulate
    """
    assert n_casc + n_ttr + n_pool + n_act == B
    nc = bacc.Bacc("TRN2", target_bir_lowering=False, debug=False)

    enc = nc.dram_tensor("enc", [S_CORE, B, H], FP16, kind="ExternalInput")
    hidb = nc.dram_tensor("hidb", [1, BH], FP16, kind="ExternalInput")
    seed = nc.dram_tensor("seed", [1, 8], F32, kind="ExternalInput")
    ctx_raw = nc.dram_tensor("ctx_raw", [4, 4096], F32, kind="ExternalOutput")
    l_raw = nc.dram_tensor("l_raw", [1, B], F32, kind="ExternalOutput")

    enc_v = enc[:].rearrange("(t p) b h -> t p (b h)", p=P)

    EXP = mybir.ActivationFunctionType.Exp
    COPY = mybir.ActivationFunctionType.Copy

    RED_L = H // red_k  # stage-1 segment width

    with tile.TileContext(nc) as tc, ExitStack() as ctx:
        encp = ctx.enter_context(tc.tile_pool(name="encp", bufs=enc_bufs))
        tmpp = ctx.enter_context(tc.tile_pool(name="tmpp", bufs=tmp_bufs))
        scrp = ctx.enter_context(tc.tile_pool(name="scrp", bufs=small_bufs))
        smallp = ctx.enter_context(tc.tile_pool(name="smallp", bufs=small_bufs))
        singles = ctx.enter_context(tc.tile_pool(name="singles", bufs=1))
        psump = ctx.enter_context(tc.tile_pool(name="psump", bufs=1, space="PSUM"))

        hidB = singles.tile([P, BH], FP16)
        h_ap = hidb[:]
        hid_bcast = bass.AP(
            tensor=h_ap.tensor, offset=h_ap.offset, ap=[[0, P], [1, BH]]
        )
        nc.gpsimd.dma_start(out=hidB[:], in_=hid_bcast)

        seed_sb = singles.tile([1, 8], F32)
        nc.sync.dma_start(out=seed_sb[:], in_=seed[:])

        neg_shift = singles.tile([P, 1], F32)
        nc.vector.memset(neg_shift[:], -EXP_SHIFT)
        ones_col = singles.tile([P, 1], BF16)
        nc.vector.memset(ones_col[:], 1.0)

        ctx_ps = psump.tile([P, 4096], F32)
        nc.vector.memset(ctx_ps[:], 0.0)

        loop_cm = tc.For_i(0, hw_loop) if hw_loop else nullcontext()
        with loop_cm:
            for rt in range(repeat * NTILES):
                r, t = divmod(rt, NTILES)
                enc_t = encp.tile([P, BH], FP16, tag="enc")
                nc.sync.dma_start(out=enc_t[:], in_=enc_v[t])

                scores_t = smallp.tile([P, B], F32, tag="scores")

                # --- pool-mul cols first (Pool starts early) ---
                pool_scr = []
                for b in range(n_casc + n_ttr, n_casc + n_ttr + n_pool):
                    pscr = scrp.tile([P, H], FP16, tag="pscr")
                    nc.gpsimd.tensor_mul(
                        pscr[:],
                        enc_t[:, b * H:(b + 1) * H],
                        hidB[:, b * H:(b + 1) * H],
                    )
                    pool_scr.append((b, pscr))

                # --- act cols: DVE bulk mul first (ACT starts earliest) ---
                act_lo = n_casc + n_ttr + n_pool
                if n_act:
                    atmp = tmpp.tile([P, n_act * H], FP16, tag="atmp")
                    nc.vector.tensor_mul(
                        atmp[:],
                        enc_t[:, act_lo * H:(act_lo + n_act) * H],
                        hidB[:, act_lo * H:(act_lo + n_act) * H],
                    )

                # --- casc cols: big mul + segmented reduce ---
                if n_casc:
                    nb = n_casc
                    tmp = tmpp.tile([P, nb * H], FP16, tag="tmp")
                    cw = nb * H // mul_split
                    for m in range(mul_split):
                        nc.vector.tensor_mul(
                            tmp[:, m * cw:(m + 1) * cw],
                            enc_t[:, m * cw:(m + 1) * cw],
                            hidB[:, m * cw:(m + 1) * cw],
                        )
                    part = scrp.tile([P, nb * red_k], FP16, tag="part")
                    with nc.allow_low_precision(
                        reason="fp16 partials over <=32-wide unit-normal "
                        "product groups; |partial| <~ 30, rounding ~2e-3 "
                        "absolute on scores of std 16"
                    ):
                        nc.vector.tensor_reduce(
                            out=part[:].rearrange(
                                "p (b k) -> p b k", k=red_k),
                            in_=tmp[:].rearrange(
                                "p (b k l) -> p (b k) l", k=red_k, l=RED_L),
                            axis=mybir.AxisListType.X,
                            op=mybir.AluOpType.add,
                        )
                    nc.vector.tensor_reduce(
                        out=scores_t[:, 0:nb],
                        in_=part[:].rearrange("p (b k) -> p b k", k=red_k),
                        axis=mybir.AxisListType.X,
                        op=mybir.AluOpType.add,
                    )

                # --- ttr cols ---
                for b in range(n_casc, n_casc + n_ttr):
                    scr = scrp.tile([P, H], FP16, tag="scr")
                    nc.vector.affine_mul_reduce(
                        out=scr[:],
                        accum_out=scores_t[:, b:b + 1],
                        in0=enc_t[:, b * H:(b + 1) * H],
                        in1=hidB[:, b * H:(b + 1) * H],
                        scale=1.0,
                        bias=0.0,
                    )

                # --- ACT accumulates for pool/act cols ---
                for b, pscr in pool_scr:
                    ascr = scrp.tile([P, H], FP16, tag="ascr")
                    nc.scalar.activation(
                        out=ascr[:],
                        in_=pscr[:],
                        func=COPY,
                        accum_out=scores_t[:, b:b + 1],
                    )
                for j in range(n_act):
                    b = act_lo + j
                    ascr = scrp.tile([P, H], FP16, tag="ascr")
                    nc.scalar.activation(
                        out=ascr[:],
                        in_=atmp[:, j * H:(j + 1) * H],
                        func=COPY,
                        accum_out=scores_t[:, b:b + 1],
                    )

                # --- exp ---
                w_t = smallp.tile([P, B], BF16, tag="w")
                for g0 in range(0, B, exp_group):
                    g1 = min(g0 + exp_group, B)
                    nc.scalar.activation(
                        out=w_t[:, g0:g1],
                        in_=scores_t[:, g0:g1],
                        func=EXP,
                        bias=-EXP_SHIFT if exp_bias_float else neg_shift[:],
                        scale=1.0,
                    )

                # --- ctx matmuls + l matmul ---
                first = rt == 0 and not hw_loop
                last = rt == repeat * NTILES - 1
                for b in range(B):
                    pb = 32 * (b % 4)
                    nc.tensor.matmul(
                        ctx_ps[pb:pb + 1, (b // 4) * 512:(b // 4) * 512 + H],
                        lhsT=w_t[:, b:b + 1],
                        rhs=enc_t[:, b * H:(b + 1) * H],
                        start=first,
                        stop=last,
                        tile_position=(0, pb),
                        skip_group_check=True,
                    )
                nc.tensor.matmul(
                    ctx_ps[1:2, 256:256 + B],
                    lhsT=ones_col[:],
                    rhs=w_t[:],
                    start=first,
                    stop=last,
                    tile_position=(0, 0),
                    skip_group_check=True,
                )

        # drain PSUM -> SBUF (ACT copy; DMA cannot read PSUM) -> HBM
        ctx_sb = singles.tile([P, 4096], F32)
        nc.scalar.copy(ctx_sb[0:97, :], ctx_ps[0:97, :])
        for g in range(4):
            nc.sync.dma_start(
                out=ctx_raw[g:g + 1, :], in_=ctx_sb[32 * g:32 * g + 1, :]
            )
        nc.sync.dma_start(out=l_raw[:], in_=ctx_sb[1:2, 256:256 + B])

    nc.compile()
    return nc


# Active configuration: builder + kwargs used by kernel() and the timing
# harness (test.py --time calls _build_current).
# Builder 1 with ttr2x: all 32 score columns via the TTR2C custom DVE op
# running in the hand-authored 2X_1PORT slot (~179ns/col vs 313 at 1x),
# scores read as the scan's last element (the 2x accumulator readout is
# broken in firmware; the running-sum output path is HW-validated exact).
# hp=258 keeps every operand pair-aligned so the engine never falls back
# to 1x. Fallback proven config: {"builder": 1, "kwargs": {}} (~70.6us).
# Measured: enc_bufs=4 + exp_group=32 together regress to 62148ns.
# Isolating enc_bufs=4 (targets the ~150ns/tile DMA trigger bubbles).
_CONFIG = {
    "builder": 1,
    "kwargs": dict(
        tile_modes=",".join(["ttr"] * 8), ttr2x=True, hp=258,
        enc_bufs=4,
    ),
}


def _build_current(**extra):
    kw = dict(_CONFIG["kwargs"])
    kw.update(extra)
    if _CONFIG["builder"] == 2:
        return _build_nc2(**kw)
    return _build_nc(**kw)


_NC_CACHE = {}


def _get_nc():
    if "nc" not in _NC_CACHE:
        _NC_CACHE["nc"] = _build_current()
    return _NC_CACHE["nc"]


def _augment_enc(enc_slice: np.ndarray, hp: int = HP) -> np.ndarray:
    """[S_CORE, B, H] f32 -> [S_CORE, B, hp] fp16: ones col at H, zero pad."""
    out = np.zeros((S_CORE, B, hp), dtype=FP16_NP)
    out[:, :, :H] = enc_slice
    out[:, :, H] = 1.0
    return out


def _prep_enc(enc_slice: np.ndarray) -> np.ndarray:
    """Per-core enc input for the active builder."""
    if _CONFIG["builder"] == 2:
        return np.ascontiguousarray(enc_slice, dtype=FP16_NP)
    return _augment_enc(enc_slice, _CONFIG["kwargs"].get("hp", HP))


def kernel(enc_output_i: np.ndarray, enc_or_dec_hid_i: np.ndarray) -> np.ndarray:
    enc = np.asarray(enc_output_i, dtype=np.float32)
    hid = np.asarray(enc_or_dec_hid_i, dtype=np.float32)[0]  # [B, H]

    hidb = np.ascontiguousarray(hid.reshape(1, BH)).astype(FP16_NP)

    nc = _get_nc()
    zseed = np.zeros((1, 8), dtype=np.float32)
    in_maps = [
        {
            "enc": _prep_enc(enc[c * S_CORE:(c + 1) * S_CORE]),
            "hidb": hidb,
            "seed": zseed,
        }
        for c in range(NCORES)
    ]
    results = run_bass_kernel_spmd(nc, in_maps, core_ids=list(range(NCORES))).results

    ctx_sum = np.zeros((B, H), dtype=np.float64)
    l_sum = np.zeros((B,), dtype=np.float64)
    for c in range(NCORES):
        raw = results[c]["ctx_raw"]  # [4, 4096]; row = b%4, col block b//4
        g = raw.reshape(4, 8, 512)
        g = np.transpose(g, (1, 0, 2)).reshape(B, 512)  # [b, 512]
        ctx_sum += g[:, :H]
        if _CONFIG["builder"] == 2:
            l_sum += results[c]["l_raw"][0]
        else:
            l_sum += g[:, H]
    out = (ctx_sum / l_sum[:, None]).astype(np.float32)
    return out

